# revision 60
# baseline (speedup 1.0000x reference)
"""Trainium2 Bass kernel for nn_MixedAttention.

Full inputs in, full output out. Sharding: 8 cores = 2 (batch) x 4 (head
pairs). Each core computes 2 global + 2 local heads for one batch element.

Key algebraic rewrite for the local branch:
    lscores = (lq@lk1^T)@(lk1@lk2^T) = lq @ (lk1^T@lk1) @ lk2^T
with M = lk1^T@lk1 a [64,64] matrix -- turns a 2048^3 matmul chain into
two small matmuls plus one S x S matmul (30x less PE work).

Precision strategy (all-bf16, no fp32/f32r matmuls):
  bf16 matmuls run at 1 cycle/column on the PE vs 4 for fp32. Paths whose
  error is not exp-amplified (global q/k/v, both value paths) run single
  bf16. The local score chain feeds exp() with raw scores up to ~5000, so
  it needs ~19 mantissa bits; we get them from bf16 hi/lo pairs
  (x = hi + lo, both bf16) with the three-term product
  hi*hi + hi*lo + lo*hi; the dropped lo*lo term is ~2^-18 relative. For
  the final S x S score matmul the hi*hi term shares a K=65 contraction
  with the -max row and the two cross terms concatenate into one K=128
  contraction, so the whole thing is 2 matmuls per tile (vs 1 fp32 matmul
  at 4x the per-column cost). Numpy end-to-end emulation: rel err 2.9e-3
  against the fp32 reference (gate is 2e-2); hardware matches at 2.86e-3.

  The softmax max-offset needs almost no precision (any per-row offset
  cancels between the ctx numerator and the ones-column denominator), so
  pass 1 computes the row max from hi-only bf16 scores and stores it in
  bf16: the offset stays within ~+-30 of the true max vs an exp-overflow
  window of +-85.

Scheduling: after a PE-dense projection phase (wq and half of wk
consume hid chunks as their DMAs land; the other projections are
back-to-back accumulation chains) and an interleaved transpose/prep phase, attention runs as four
fused units per (head x i-half pair). Each unit's jc step does: filler
pass-1 pieces first (their fast tensor_tensor-max consumers free PSUM
slots quickly, giving the slow exp-consumer stt allocations a full step
of slack in the 2-slot ring), then the score matmuls, the [128,1024]
exp on ACT, and the ctx accumulation for e[jc-2]. The two global-head
units carry all of pass-1; the two local-head units run two exp lanes
in one loop (halving the per-lane ACT-queue floor of ~1.7us/exp). ctx
PSUM is evacuated by the vector engine: the scalar queue feeds the next
unit's exps, so evacuations there stalled each unit handoff by ~3us.
Row maxes bounce through DRAM to become the A-tiles' row 64.

Measured: 460 us on hardware (vs 645 us for the fp32/f32r baseline),
85-88% tensor-engine occupancy; a ~200 us chip power-throttle window
(K=4/8 half clock, firmware period) is the main residual cost -- the
PE streams with <8 us of total gaps through the throttled region.

PSUM budget (8 banks): stt ring 2x[128,1024] (4) + "ctx" pair (2) +
"sc" pair (2); the sc/ctx rings multiplex projection accumulators,
M/qm accumulators, transposes, pass-1 score tiles, and the second
lane's ctx accumulators in merged units (which therefore take no
fillers).
"""

import math
import os
import sys

import numpy as np

sys.path.insert(0, "/opt/trn_rl_repo")

B, S, HID, HEAD = 2, 2048, 1024, 64
SC = S // 128  # 16 s-chunks of 128
HC = HID // 128  # 8 hidden chunks
N_CORES = 8
SCALE = 1.0 / math.sqrt(HEAD)

W_SINGLE = ["wq", "wk", "wv", "wlv"]  # single-bf16 projections
W_HILO = ["wlq", "wlk1", "wlk2"]  # hi/lo bf16 projections
W_ALL = W_SINGLE + W_HILO

_CACHE = {}
LAST_RESULTS = None  # stash of BassKernelResults for test.py profiling


def _build():
    import concourse.mybir as mybir
    import concourse.tile as tile
    from concourse import bacc
    from concourse.masks import make_identity

    f32 = mybir.dt.float32
    bf16 = mybir.dt.bfloat16
    AF = mybir.ActivationFunctionType
    ALU = mybir.AluOpType
    AX = mybir.AxisListType

    nc = bacc.Bacc("TRN2", target_bir_lowering=False, debug=False,
                   enable_asserts=False)

    hid_hi_d = nc.dram_tensor("hid_hi", (HID, S), bf16,
                              kind="ExternalInput").ap()
    hid_lo_d = nc.dram_tensor("hid_lo", (HID, S), bf16,
                              kind="ExternalInput").ap()
    mask_d = nc.dram_tensor("mask", (S,), f32, kind="ExternalInput").ap()
    w_d = {}
    for n in W_SINGLE:
        w_d[n] = nc.dram_tensor(n, (HID, 128), bf16, kind="ExternalInput").ap()
    for n in W_HILO:
        for p in ("hi", "lo"):
            w_d[f"{n}_{p}"] = nc.dram_tensor(
                f"{n}_{p}", (HID, 128), bf16, kind="ExternalInput").ap()
    bias_d = nc.dram_tensor("biases", (len(W_ALL), 128), f32,
                            kind="ExternalInput").ap()
    out_d = nc.dram_tensor("out", (S, 256), f32, kind="ExternalOutput").ap()

    with tile.TileContext(nc) as tc:
        with (
            tc.tile_pool(name="const", bufs=1) as constp,
            tc.tile_pool(name="persist", bufs=1) as pp,
            tc.tile_pool(name="hid", bufs=1) as hp,
            tc.tile_pool(name="io", bufs=5) as iop,
            tc.tile_pool(name="wp", bufs=1) as wp,
            tc.tile_pool(name="epool", bufs=6) as ep,
            tc.tile_pool(name="opool", bufs=1) as op_,
            tc.tile_pool(name="ps_big", bufs=2, space="PSUM") as ps_big,
            tc.tile_pool(name="ps_ctx", bufs=2, space="PSUM") as ps_ctx,
            tc.tile_pool(name="dramp", bufs=2, space="DRAM") as dramp,
        ):
            ident = constp.tile([128, 128], f32, name="ident")
            make_identity(nc, ident)
            identb = constp.tile([128, 128], bf16, name="identb")
            nc.vector.tensor_copy(identb, ident)
            biases_sb = constp.tile([128, len(W_ALL)], f32, name="biases_sb")
            mask_sb = constp.tile([128, SC], f32, name="mask_sb")
            bias_sb = {n: biases_sb[:, i:i + 1]
                       for i, n in enumerate(W_ALL)}

            # persistent projection outputs [channel, s] (2 heads packed)
            projT = {n: pp.tile([128, S], bf16, name=f"projT_{n}")
                     for n in ("wq", "wk", "wv", "wlv")}
            lqT = {p: pp.tile([128, S], bf16, name=f"lqT_{p}")
                   for p in ("hi", "lo")}
            lk1T = {p: pp.tile([128, S], bf16, name=f"lk1T_{p}")
                    for p in ("hi", "lo")}
            lk2T = {p: pp.tile([128, S], bf16, name=f"lk2T_{p}")
                    for p in ("hi", "lo")}

            out_sb = op_.tile([128, SC, 256], f32, name="out_sb")

            hid_hi = hp.tile([128, HC, S], bf16, name="hid_hi")
            hid_lo = hp.tile([128, HC, S], bf16, name="hid_lo")
            hh_r = hid_hi_d.rearrange("(c p) s -> p c s", p=128)
            hl_r = hid_lo_d.rearrange("(c p) s -> p c s", p=128)
            wsb = {}

            def wdma(key):
                # io ring (bufs=5): later weights reuse earlier slots once
                # the earlier projection's matmuls are done
                wsb[key] = iop.tile([128, HC, 128], bf16, tag="w",
                                    name=f"w_{key}")
                nc.gpsimd.dma_start(
                    wsb[key], w_d[key].rearrange("(c p) m -> p c m", p=128))

            # startup-latency-aware DMA order: hid_hi is split across
            # both queues so the hc-outer wq sweep can consume chunks as
            # they land; ring-reusing w DMAs (which block the gpsimd
            # queue until their slot frees) go last
            wdma("wq")
            for hc in range(HC):
                eng = nc.sync if hc % 2 == 0 else nc.gpsimd
                eng.dma_start(hid_hi[:, hc], hh_r[:, hc])
            wdma("wk")
            wdma("wv")
            nc.gpsimd.dma_start(biases_sb, bias_d.rearrange("n p -> p n"))
            for hc in range(HC):
                eng = nc.sync if hc % 2 == 1 else nc.gpsimd
                eng.dma_start(hid_lo[:, hc], hl_r[:, hc])
            nc.sync.dma_start(mask_sb,
                              mask_d.rearrange("(c p) -> p c", p=128))
            wdma("wlv")
            wdma("wlk1_hi")
            wdma("wlk1_lo")
            wdma("wlq_hi")
            wdma("wlq_lo")
            wdma("wlk2_hi")
            wdma("wlk2_lo")

            # ---------- projection piece generators ----------
            # a "piece" is ~8 matmuls (one HC sweep); hi/lo projections
            # are 3 consecutive pieces sharing one accumulator

            def proj_single_pieces(n, dst):
                pieces = []
                state = {}
                for ic in range(4):
                    def p(ic=ic):
                        isl = slice(ic * 512, (ic + 1) * 512)
                        acc = ps_big.tile([128, 512], f32, tag="big",
                                         name="acc")
                        for hc in range(HC):
                            nc.tensor.matmul(
                                acc, lhsT=wsb[n][:, hc],
                                rhs=hid_hi[:, hc, isl],
                                start=(hc == 0), stop=(hc == HC - 1))
                        nc.vector.tensor_scalar_add(
                            dst[:, isl], acc, bias_sb[n])
                    pieces.append(p)
                return pieces

            def proj_hilo_pieces(n, dhi, dlo):
                pieces = []
                state = {}
                for ic in range(4):
                    isl = slice(ic * 512, (ic + 1) * 512)

                    def pa(isl=isl):
                        acc = ps_big.tile([128, 512], f32, tag="big",
                                         name="acc")
                        state["acc"] = acc
                        for hc in range(HC):
                            nc.tensor.matmul(
                                acc, lhsT=wsb[f"{n}_hi"][:, hc],
                                rhs=hid_hi[:, hc, isl],
                                start=(hc == 0), stop=False)

                    def pb(isl=isl):
                        acc = state["acc"]
                        for hc in range(HC):
                            nc.tensor.matmul(
                                acc, lhsT=wsb[f"{n}_lo"][:, hc],
                                rhs=hid_hi[:, hc, isl],
                                start=False, stop=False)

                    def pc(isl=isl):
                        acc = state["acc"]
                        for hc in range(HC):
                            nc.tensor.matmul(
                                acc, lhsT=wsb[f"{n}_hi"][:, hc],
                                rhs=hid_lo[:, hc, isl],
                                start=False, stop=(hc == HC - 1))
                        nc.vector.tensor_scalar_add(
                            dhi[:, isl], acc, bias_sb[n])
                        # lo = (acc + bias) - hi, exact in fp32 then bf16
                        nc.vector.scalar_tensor_tensor(
                            dlo[:, isl], acc, bias_sb[n], dhi[:, isl],
                            op0=ALU.add, op1=ALU.subtract)
                    pieces += [pa, pb, pc]
                return pieces

            # ---------- attention helpers ----------

            def build_vaug(vT):
                # v natural [s, d] + ones column -> [128, SC, 65] bf16
                base = vT.base_partition()
                idsl = slice(base, base + 64)
                vaug = wp.tile([128, SC, 65], bf16, tag="vaug",
                               name="vaug", bufs=4)
                nc.gpsimd.memset(vaug[:, :, 64], 1.0)
                for t in range(SC):
                    pt = ps_ctx.tile([128, 128], bf16, tag="sc", name="pt")
                    nc.tensor.transpose(
                        pt[:, :64], vT[:, t * 128:(t + 1) * 128],
                        identb[idsl, idsl])
                    nc.vector.tensor_copy(vaug[:, t, :64], pt[:, :64])
                return vaug

            def attn_unit(lanes, fillers, k):
                # fused: scores -> exp -> ctx accumulation, with up to k
                # filler pieces popped per jc step. Each lane is
                # (head, vaug, ihalf, st_emit); a merged (2-lane) unit
                # shares one loop so the ACT queue cost per lane halves.
                # Lane 0's accumulators take the held "ctx" slots; lane
                # 1's take the "sc" scratch slots (merged units must run
                # with k=0 so nothing else allocates "sc" mid-loop).
                ctxs = []
                for li, _ in enumerate(lanes):
                    tag = "ctx" if li == 0 else "sc"
                    ctxs.append([ps_ctx.tile([65, 512], f32, tag=tag,
                                             name="ctx")
                                 for _ in range(2)])
                es = [[] for _ in lanes]

                def ctx_mms(jd):
                    for li, (head, vaug, ihalf, st_emit) in \
                            enumerate(lanes):
                        for ic2 in range(2):
                            nc.tensor.matmul(
                                ctxs[li][ic2], lhsT=vaug[:, jd],
                                rhs=es[li][jd][:,
                                               ic2 * 512:(ic2 + 1) * 512],
                                start=(jd == 0), stop=(jd == SC - 1))

                for jc in range(SC):
                    # filler (pass-1) allocations go FIRST: their fast
                    # TT consumers then sit between the slow exp-consumer
                    # stt allocations in the big ring, so every slot-wait
                    # has a full jc step of slack and the PE streams
                    # gaplessly (HAM stays at full clock)
                    for _ in range(k):
                        if fillers:
                            fillers.popleft()()
                    for li, (head, vaug, ihalf, st_emit) in \
                            enumerate(lanes):
                        stt = ps_big.tile([128, 1024], f32, tag="big",
                                          name="stt")
                        bias = st_emit(stt, jc, ihalf)
                        e = ep.tile([128, 1024], bf16, tag="e", name="e")
                        nc.scalar.activation(e, stt, AF.Exp, bias=bias,
                                             scale=SCALE)
                        es[li].append(e)
                    if jc >= 2:
                        ctx_mms(jc - 2)
                ctx_mms(SC - 2)
                ctx_mms(SC - 1)

                # evacuate all accumulators first (scalar engine, frees
                # the psum slots), then transpose/scale/store
                sbcs = {}
                for li, (head, vaug, ihalf, st_emit) in enumerate(lanes):
                    for ic2 in range(2):
                        ctx_sbc = wp.tile([65, 512], f32, tag="ctx_sbc",
                                          name="ctx_sbc", bufs=4)
                        nc.vector.tensor_copy(ctx_sbc, ctxs[li][ic2])
                        sbcs[(li, ic2)] = ctx_sbc
                for li, (head, vaug, ihalf, st_emit) in enumerate(lanes):
                    csl = slice(head * 64, (head + 1) * 64)
                    for ic2 in range(2):
                        ic = ihalf * 2 + ic2
                        ctx_sbc = sbcs[(li, ic2)]
                        for tt in range(4):
                            t = ic * 4 + tt
                            pt = ps_ctx.tile([128, 128], f32,
                                             tag="sc" if tt % 2 else "ctx",
                                             name="ptf")
                            nc.tensor.transpose(
                                pt[:, :65],
                                ctx_sbc[:, tt * 128:(tt + 1) * 128],
                                ident[:65, :65])
                            rec = wp.tile([128, 1], f32, tag="rec",
                                          name="rec", bufs=4)
                            nc.vector.reciprocal(rec, pt[:, 64:65])
                            nc.vector.tensor_scalar_mul(
                                out_sb[:, t, csl], pt[:, :64], rec)
                        nc.sync.dma_start(
                            out_d.rearrange("(t p) c -> p t c", p=128)[
                                :, ic * 4:(ic + 1) * 4, csl],
                            out_sb[:, ic * 4:(ic + 1) * 4, csl])

            def g_st_emit(kT, qT):
                def emit(stt, jc, ihalf):
                    jsl = slice(jc * 128, (jc + 1) * 128)
                    for h2 in range(2):
                        i0 = ihalf * 1024 + h2 * 512
                        nc.tensor.matmul(
                            stt[:, h2 * 512:(h2 + 1) * 512],
                            lhsT=kT[:, jsl], rhs=qT[:, i0:i0 + 512],
                            start=True, stop=True)
                    return mask_sb[:, jc:jc + 1]
                return emit

            def l_st_emit(hs):
                A, Bt, C, D = hs["A"], hs["B"], hs["C"], hs["D"]

                def emit(stt, jc, ihalf):
                    jsl = slice(jc * 128, (jc + 1) * 128)
                    for h2 in range(2):
                        i0 = ihalf * 1024 + h2 * 512
                        out = stt[:, h2 * 512:(h2 + 1) * 512]
                        nc.tensor.matmul(out, lhsT=Bt[:, jsl],
                                         rhs=A[:, i0:i0 + 512],
                                         start=True, stop=False)
                        nc.tensor.matmul(out, lhsT=D[:, jsl],
                                         rhs=C[:, i0:i0 + 512],
                                         start=False, stop=True)
                    return 0.0
                return emit

            def local_prep_a(hh):
                # lk1 natural [s, d] hi/lo via transposes, then
                # M = lk1^T @ lk1 (three-term) and its hi/lo split
                rs = slice(hh * 64, (hh + 1) * 64)
                base = hh * 64
                idsl = slice(base, base + 64)
                lk1nat = {}
                for p in ("hi", "lo"):
                    nat = wp.tile([128, SC, 64], bf16, tag=f"lk1nat{p}",
                                  name=f"lk1nat_{p}", bufs=1)
                    for t in range(SC):
                        pt = ps_ctx.tile([128, 128], bf16, tag="sc",
                                        name="pt")
                        nc.tensor.transpose(
                            pt[:, :64],
                            lk1T[p][rs, t * 128:(t + 1) * 128],
                            identb[idsl, idsl])
                        nc.vector.tensor_copy(nat[:, t], pt[:, :64])
                    lk1nat[p] = nat

                mps = ps_ctx.tile([65, 512], f32, tag="ctx", name="mps")
                for t in range(SC):
                    nc.tensor.matmul(mps[:64, :64],
                                     lhsT=lk1nat["hi"][:, t],
                                     rhs=lk1nat["hi"][:, t],
                                     start=(t == 0), stop=False)
                for t in range(SC):
                    nc.tensor.matmul(mps[:64, :64],
                                     lhsT=lk1nat["lo"][:, t],
                                     rhs=lk1nat["hi"][:, t],
                                     start=False, stop=False)
                for t in range(SC):
                    nc.tensor.matmul(mps[:64, :64],
                                     lhsT=lk1nat["hi"][:, t],
                                     rhs=lk1nat["lo"][:, t],
                                     start=False, stop=(t == SC - 1))
                m_hi = wp.tile([64, 64], bf16, tag="m_hi", name="m_hi",
                               bufs=2)
                m_lo = wp.tile([64, 64], bf16, tag="m_lo", name="m_lo",
                               bufs=2)
                nc.vector.tensor_copy(m_hi, mps[:64, :64])
                nc.vector.tensor_sub(m_lo, mps[:64, :64], m_hi)
                if hh == 0:
                    return dict(m_hi=m_hi, m_lo=m_lo)
                # head 1 contracts on partitions 64-127: shift M there so
                # lhsT/rhs share a partition base (DMA can cross
                # partitions; compute engines cannot)
                m_hi2 = wp.tile([128, 64], bf16, tag="m_hi2", name="m_hi2")
                m_lo2 = wp.tile([128, 64], bf16, tag="m_lo2", name="m_lo2")
                nc.sync.dma_start(m_hi2[64:128], m_hi)
                nc.sync.dma_start(m_lo2[64:128], m_lo)
                return dict(m_hi=m_hi2[64:128], m_lo=m_lo2[64:128])

            def local_prep_b(hh, mm):
                # B/D operand tiles from lk2, then qm = M @ lq^T
                # (three-term) split into A/C, plus the head's vaug
                rs = slice(hh * 64, (hh + 1) * 64)
                Bt = wp.tile([65, S], bf16, tag="B", name="Bt", bufs=2)
                D = wp.tile([128, S], bf16, tag="D", name="D", bufs=2)
                if hh == 0:
                    nc.vector.tensor_copy(D[:64], lk2T["lo"][rs])
                    nc.sync.dma_start(D[64:128], lk2T["hi"][rs])
                    nc.scalar.copy(Bt[:64], lk2T["hi"][rs])
                else:
                    nc.sync.dma_start(D[:64], lk2T["lo"][rs])
                    nc.vector.tensor_copy(D[64:128], lk2T["hi"][rs])
                    nc.gpsimd.dma_start(Bt[:64], lk2T["hi"][rs])
                nc.gpsimd.memset(Bt[64:65], 1.0)

                A = wp.tile([65, S], bf16, tag="A", name="A", bufs=2)
                C = wp.tile([128, S], bf16, tag="C", name="C", bufs=2)
                lo_tmp = wp.tile([64, S], bf16, tag="lo_tmp",
                                 name="lo_tmp", bufs=1)
                lq_hi, lq_lo = lqT["hi"][rs], lqT["lo"][rs]
                for ic in range(4):
                    isl = slice(ic * 512, (ic + 1) * 512)
                    acc = ps_ctx.tile([65, 512], f32, tag="ctx",
                                      name="qacc")
                    nc.tensor.matmul(acc[:64], lhsT=mm["m_hi"],
                                     rhs=lq_hi[:, isl],
                                     start=True, stop=False)
                    nc.tensor.matmul(acc[:64], lhsT=mm["m_lo"],
                                     rhs=lq_hi[:, isl],
                                     start=False, stop=False)
                    nc.tensor.matmul(acc[:64], lhsT=mm["m_hi"],
                                     rhs=lq_lo[:, isl],
                                     start=False, stop=True)
                    nc.vector.tensor_copy(C[:64, isl], acc[:64])
                    nc.vector.tensor_sub(lo_tmp[:, isl], acc[:64],
                                         C[:64, isl])
                nc.sync.dma_start(C[64:128], lo_tmp)
                nc.scalar.copy(A[:64], C[:64])

                vaug = build_vaug(projT["wlv"][rs])
                return dict(A=A, B=Bt, C=C, D=D, vaug=vaug)

            def pass1_pieces(hs, maxneg, mscr):
                # hi-only scores, natural [i, j] orientation. The max path
                # avoids the vector engine entirely: the scalar engine
                # evacuates each score tile to SBUF bf16 (freeing the psum
                # slot fast -- a DVE reduce-in-place made the PE->DVE->PE
                # round trip the unit pacer), and the otherwise-idle
                # gpsimd engine does the max reduces from SBUF. bf16
                # scores/max are plenty (the exp window is +-85). The
                # final DMA bounce relayouts -max into A row 64 via DRAM.
                C, Bt, A = hs["C"], hs["B"], hs["A"]
                pieces = []
                accs = {}
                for t in range(SC):
                    for j4 in range(4):
                        def p(t=t, j4=j4):
                            tsl = slice(t * 128, (t + 1) * 128)
                            st1 = ps_big.tile([128, 512], f32,
                                              tag="big", name="st1")
                            nc.tensor.matmul(
                                st1, lhsT=C[:64, tsl],
                                rhs=Bt[:64, j4 * 512:(j4 + 1) * 512],
                                start=True, stop=True)
                            if j4 == 0:
                                # elementwise max-accumulate: each TT op
                                # is ~270ns and frees the psum slot fast
                                # (a reduce-in-place held it ~700ns and
                                # made PE->DVE->PE the unit pacer)
                                acc = wp.tile([128, 512], f32,
                                              tag="p1acc", name="p1acc",
                                              bufs=1)
                                accs[t] = acc
                                nc.vector.tensor_copy(acc, st1)
                            else:
                                nc.vector.tensor_max(accs[t], st1,
                                                     accs[t])
                            if j4 == 3:
                                nc.vector.tensor_reduce(
                                    maxneg[:, t:t + 1], accs[t],
                                    axis=AX.X, op=ALU.max,
                                    negate=True)
                                if t == SC - 1:
                                    nc.sync.dma_start(
                                        mscr.rearrange("(t p) -> p t",
                                                       p=128), maxneg)
                                    nc.sync.dma_start(A[64:65, :],
                                                      mscr[None, :])
                        pieces.append(p)
                return pieces

            # ---------- emission schedule ----------
            from collections import deque

            # wq (and half of wk) run hc-outer, consuming hid chunks as
            # their DMAs land (ic-outer would stall on the LAST chunk
            # before any chain finishes); 6 accumulators live across the
            # sweep so the DMA-paced window carries ~1.3us of PE work
            # per 1.6us chunk arrival
            wq_accs = [ps_ctx.tile([128, 512], f32, tag=t, name="wqacc")
                       for t in ("ctx", "ctx", "sc", "sc")]
            wk_accs = [ps_big.tile([128, 512], f32, tag="big",
                                   name="wkacc") for _ in range(2)]
            for hc in range(HC):
                for ic in range(4):
                    nc.tensor.matmul(
                        wq_accs[ic], lhsT=wsb["wq"][:, hc],
                        rhs=hid_hi[:, hc, ic * 512:(ic + 1) * 512],
                        start=(hc == 0), stop=(hc == HC - 1))
                for ic in range(2):
                    nc.tensor.matmul(
                        wk_accs[ic], lhsT=wsb["wk"][:, hc],
                        rhs=hid_hi[:, hc, ic * 512:(ic + 1) * 512],
                        start=(hc == 0), stop=(hc == HC - 1))
            for ic in range(4):
                nc.vector.tensor_scalar_add(
                    projT["wq"][:, ic * 512:(ic + 1) * 512],
                    wq_accs[ic], bias_sb["wq"])
            for ic in range(2):
                nc.vector.tensor_scalar_add(
                    projT["wk"][:, ic * 512:(ic + 1) * 512],
                    wk_accs[ic], bias_sb["wk"])
            # rest of wk, then wv, plain (chunks all resident by now)
            for piece in proj_single_pieces("wk", projT["wk"])[2:]:
                piece()
            for piece in proj_single_pieces("wv", projT["wv"]):
                piece()

            # (wlv emitted below, weaving the wv-based vaug transposes)

            # interleaved transpose phase: 8 independent streams
            # alternating the sc and ctx rings (depth 4 total) hide the
            # per-transpose evacuation latency
            lk1nat = {}
            vaugs = {}
            tr_jobs = []
            for hh in range(2):
                rs = slice(hh * 64, (hh + 1) * 64)
                idsl = slice(hh * 64, hh * 64 + 64)
                for p in ("hi", "lo"):
                    nat = wp.tile([128, SC, 64], bf16,
                                  tag=f"lk1nat{p}{hh}",
                                  name=f"lk1nat_{p}{hh}")
                    lk1nat[(hh, p)] = nat
                    tr_jobs.append((lk1T[p][rs], idsl, nat))
            for vi, vT in enumerate((projT["wv"][0:64],
                                     projT["wv"][64:128],
                                     projT["wlv"][0:64],
                                     projT["wlv"][64:128])):
                base = vT.base_partition()
                idsl = slice(base, base + 64)
                vaug = wp.tile([128, SC, 65], bf16, tag="vaug",
                               name="vaug", bufs=4)
                nc.gpsimd.memset(vaug[:, :, 64], 1.0)
                vaugs[vi] = vaug
                tr_jobs.append((vT, idsl, vaug[:, :, :64]))
            def tr_piece(src_t, idsl, dst, t, k):
                def piece():
                    pt = ps_ctx.tile([128, 128], bf16,
                                     tag="sc" if k % 2 else "ctx",
                                     name="pt")
                    nc.tensor.transpose(
                        pt[:, :64], src_t[:, t * 128:(t + 1) * 128],
                        identb[idsl, idsl])
                    if k % 4 < 2:
                        nc.vector.tensor_copy(dst[:, t], pt[:, :64])
                    else:
                        nc.scalar.copy(dst[:, t], pt[:, :64])
                return piece

            # projection accumulators live on the big ring, so woven
            # transposes get all four sc+ctx slots with dual-engine
            # evacuation -- the PE alternates dense projection chains
            # with transpose pairs at no stream stall
            vaug_tr = [tr_piece(*tr_jobs[ji], t, ji * SC + t)
                       for ji in range(4, 8) for t in range(SC)]
            lk1_tr = [tr_piece(*tr_jobs[ji], t, ji * SC + t)
                      for ji in range(4) for t in range(SC)]
            for piece in proj_single_pieces("wlv", projT["wlv"]):
                piece()
                for _ in range(3):
                    if vaug_tr:
                        vaug_tr.pop(0)()
            for piece in (proj_hilo_pieces("wlq", lqT["hi"], lqT["lo"])
                          + proj_hilo_pieces("wlk1", lk1T["hi"],
                                             lk1T["lo"])):
                piece()
                for _ in range(3):
                    if vaug_tr:
                        vaug_tr.pop(0)()
            for piece in proj_hilo_pieces("wlk2", lk2T["hi"],
                                          lk2T["lo"]):
                piece()
                for _ in range(5):
                    if lk1_tr:
                        lk1_tr.pop(0)()
            for piece in vaug_tr + lk1_tr:
                piece()

            def local_mq(hh):
                # M = lk1^T @ lk1 (three-term) then qm = M @ lq^T; build
                # the per-head A/B/C/D operand tiles
                rs = slice(hh * 64, (hh + 1) * 64)
                nat_hi, nat_lo = lk1nat[(hh, "hi")], lk1nat[(hh, "lo")]
                mps = ps_ctx.tile([65, 512], f32, tag="ctx", name="mps")
                for t in range(SC):
                    nc.tensor.matmul(mps[:64, :64], lhsT=nat_hi[:, t],
                                     rhs=nat_hi[:, t],
                                     start=(t == 0), stop=False)
                for t in range(SC):
                    nc.tensor.matmul(mps[:64, :64], lhsT=nat_lo[:, t],
                                     rhs=nat_hi[:, t],
                                     start=False, stop=False)
                for t in range(SC):
                    nc.tensor.matmul(mps[:64, :64], lhsT=nat_hi[:, t],
                                     rhs=nat_lo[:, t],
                                     start=False, stop=(t == SC - 1))
                m_hi = wp.tile([64, 64], bf16, tag="m_hi", name="m_hi",
                               bufs=2)
                m_lo = wp.tile([64, 64], bf16, tag="m_lo", name="m_lo",
                               bufs=2)
                nc.vector.tensor_copy(m_hi, mps[:64, :64])
                nc.vector.tensor_sub(m_lo, mps[:64, :64], m_hi)
                if hh == 1:
                    # head 1 contracts on partitions 64-127: DMA-shift M
                    m_hi2 = wp.tile([128, 64], bf16, tag="m_hi2",
                                    name="m_hi2")
                    m_lo2 = wp.tile([128, 64], bf16, tag="m_lo2",
                                    name="m_lo2")
                    nc.sync.dma_start(m_hi2[64:128], m_hi)
                    nc.sync.dma_start(m_lo2[64:128], m_lo)
                    m_hi, m_lo = m_hi2[64:128], m_lo2[64:128]

                Bt = wp.tile([65, S], bf16, tag="B", name="Bt", bufs=2)
                D = wp.tile([128, S], bf16, tag="D", name="D", bufs=2)
                if hh == 0:
                    nc.vector.tensor_copy(D[:64], lk2T["lo"][rs])
                    nc.sync.dma_start(D[64:128], lk2T["hi"][rs])
                    nc.scalar.copy(Bt[:64], lk2T["hi"][rs])
                else:
                    nc.sync.dma_start(D[:64], lk2T["lo"][rs])
                    nc.vector.tensor_copy(D[64:128], lk2T["hi"][rs])
                    nc.gpsimd.dma_start(Bt[:64], lk2T["hi"][rs])
                nc.gpsimd.memset(Bt[64:65], 1.0)

                A = wp.tile([65, S], bf16, tag="A", name="A", bufs=2)
                C = wp.tile([128, S], bf16, tag="C", name="C", bufs=2)
                lo_tmp = wp.tile([64, S], bf16, tag="lo_tmp",
                                 name="lo_tmp", bufs=1)
                lq_hi, lq_lo = lqT["hi"][rs], lqT["lo"][rs]
                for ic in range(4):
                    isl = slice(ic * 512, (ic + 1) * 512)
                    acc = ps_ctx.tile([65, 512], f32, tag="ctx",
                                      name="qacc")
                    nc.tensor.matmul(acc[:64], lhsT=m_hi,
                                     rhs=lq_hi[:, isl],
                                     start=True, stop=False)
                    nc.tensor.matmul(acc[:64], lhsT=m_lo,
                                     rhs=lq_hi[:, isl],
                                     start=False, stop=False)
                    nc.tensor.matmul(acc[:64], lhsT=m_hi,
                                     rhs=lq_lo[:, isl],
                                     start=False, stop=True)
                    nc.vector.tensor_copy(C[:64, isl], acc[:64])
                    nc.vector.tensor_sub(lo_tmp[:, isl], acc[:64],
                                         C[:64, isl])
                nc.sync.dma_start(C[64:128], lo_tmp)
                nc.scalar.copy(A[:64], C[:64])
                return dict(A=A, B=Bt, C=C, D=D)

            st2 = local_mq(0)
            st3 = local_mq(1)
            st2["vaug"] = vaugs[2]
            st3["vaug"] = vaugs[3]

            maxneg = {}
            mscr = {}
            for i in range(2):
                maxneg[i] = wp.tile([128, SC], bf16, tag=f"maxneg{i}",
                                    name=f"maxneg{i}")
                mscr[i] = dramp.tile([S], bf16, tag=f"mscr{i}",
                                     name=f"mscr{i}")

            g_emit0 = g_st_emit(projT["wk"][0:64], projT["wq"][0:64])
            g_emit1 = g_st_emit(projT["wk"][64:128], projT["wq"][64:128])

            p1q = deque(pass1_pieces(st2, maxneg[0], mscr[0]))
            attn_unit([(0, vaugs[0], 0, g_emit0),
                       (0, vaugs[0], 1, g_emit0)], p1q, 4)
            p1q = deque(pass1_pieces(st3, maxneg[1], mscr[1]))
            attn_unit([(1, vaugs[1], 0, g_emit1),
                       (1, vaugs[1], 1, g_emit1)], p1q, 4)
            attn_unit([(2, st2["vaug"], 0, l_st_emit(st2)),
                       (2, st2["vaug"], 1, l_st_emit(st2))], deque(), 0)
            attn_unit([(3, st3["vaug"], 0, l_st_emit(st3)),
                       (3, st3["vaug"], 1, l_st_emit(st3))], deque(), 0)

    nc.compile()
    return nc


def _bf16_split(x):
    import ml_dtypes
    x = np.asarray(x, np.float32)
    hi = x.astype(ml_dtypes.bfloat16)
    lo = (x - hi.astype(np.float32)).astype(ml_dtypes.bfloat16)
    return np.ascontiguousarray(hi), np.ascontiguousarray(lo)


def kernel(**inputs):
    import ml_dtypes
    from concourse import bass_utils

    global LAST_RESULTS
    if "nc" not in _CACHE:
        _CACHE["nc"] = _build()
    nc = _CACHE["nc"]

    inputs = dict(inputs)
    inputs["wlv"] = np.asarray(inputs["wlv1"], np.float32) + \
        np.asarray(inputs["wlv2"], np.float32)
    inputs["blv"] = np.asarray(inputs["blv1"], np.float32) + \
        np.asarray(inputs["blv2"], np.float32)
    hs = np.ascontiguousarray(np.asarray(inputs["hidden_states"], np.float32))
    am = np.ascontiguousarray(np.asarray(inputs["attention_mask"], np.float32))
    in_maps = []
    for c in range(N_CORES):
        b, g = c // 4, c % 4
        csl = slice(128 * g, 128 * (g + 1))
        hhi, hlo = _bf16_split(hs[b].T)
        m = {"hid_hi": hhi, "hid_lo": hlo,
             "mask": np.ascontiguousarray(am[b, 0, 0])}
        biases = np.zeros((len(W_ALL), 128), np.float32)
        for i, n in enumerate(W_ALL):
            biases[i] = np.asarray(inputs["b" + n[1:]], np.float32)[csl]
        m["biases"] = biases
        for n in W_SINGLE:
            m[n] = np.ascontiguousarray(
                np.asarray(inputs[n], np.float32)[:, csl]
            ).astype(ml_dtypes.bfloat16)
        for n in W_HILO:
            whi, wlo = _bf16_split(np.asarray(inputs[n], np.float32)[:, csl])
            m[f"{n}_hi"], m[f"{n}_lo"] = whi, wlo
        in_maps.append(m)

    res = bass_utils.run_bass_kernel_spmd(
        nc, in_maps, list(range(N_CORES)),
        tmpdir=os.environ.get("BASS_TMPDIR"))
    LAST_RESULTS = res

    out = np.zeros((B, S, HID), np.float32)
    for c in range(N_CORES):
        b, g = c // 4, c % 4
        o = res.results[c]["out"]
        out[b, :, 128 * g:128 * (g + 1)] = o[:, :128]
        out[b, :, 512 + 128 * g:512 + 128 * (g + 1)] = o[:, 128:]
    return out
